# revision 13
# baseline (speedup 1.0000x reference)
"""Trainium2 Bass kernel for nn_DecoderNet: 27-step attention-LSTM decoder.

Key algebraic identity: the attention MLP has no nonlinearities, and softmax
is invariant to per-row constants, so the attention weights (and hence the
context vector) are constant across all 27 decode steps:
    score[b,s] = enc[b,s,:] @ u_e + const(b),  u_e = (W1 @ W2 @ W3 @ Wv)[:H]
Everything h-dependent in the score is constant over s and drops out of the
softmax.  ctx is computed once; its gate-space contribution
(ctx @ Wih[WD:] + bih + bhh) is folded into a per-batch constant.

Distribution: the recurrence (gates + LSTM elementwise) is replicated on all
8 cores; the output projection (512 x 30000) is vocab-sharded 8 ways so the
Wout slice stays resident in SBUF.  The per-step global argmax is resolved
with one small AllGather per step.  fp32 matmuls throughout (argmax margins
are ~1e-6, bf16/tf32 would flip decisions); gates/logits use col-group
tile_position pairing to run two fp32 matmuls concurrently.
"""

import numpy as np

B = 64
S = 256
H = 512
WD = 1024
V = 30000
N_CORES = 8
VSH = V // N_CORES          # 3750 real columns per core
VPAD = 4096                 # padded per-core slice in DRAM
NG = 4 * H                  # 2048 gate columns
KW = WD // 128              # 8
KH = H // 128               # 4
KX = KW + KH                # 12 contraction chunks for gates
# vg half-ranges within the 3750-wide local slice: (loL, nL, loR, nR)
VG_RANGES = [(0, 512, 512, 512), (1024, 512, 1536, 512),
             (2048, 512, 2560, 512), (3072, 512, 3584, 166)]

_CACHE = {}


def _build(T):
    import contextlib
    import concourse.bass as bass
    import concourse.tile as tile
    from concourse import bacc, mybir

    f32 = mybir.dt.float32
    u32 = mybir.dt.uint32
    i32 = mybir.dt.int32
    Alu = mybir.AluOpType
    Axis = mybir.AxisListType
    Act = mybir.ActivationFunctionType

    nc = bacc.Bacc("TRN2", target_bir_lowering=False, debug=False,
                   num_devices=N_CORES)

    def dram(n, shp, dt=f32, kind="ExternalInput"):
        return nc.dram_tensor(n, shp, dt, kind=kind).ap()

    emb_d = dram("emb", [V, WD])
    enc_d = dram("enc", [B * S, H])           # natural layout, flattened
    encT_d = dram("encT", [H, B * S])         # host-transposed
    Wp_d = dram("Wp", [KX * 128, NG])         # [Wih_w ; Whh]
    Wc_d = dram("Wc", [H, NG])                # Wih rows WD:WD+H (ctx part)
    W1eT_d = dram("W1eT", [H, H])             # W1[:H,:].T
    W2T_d = dram("W2T", [H, H])
    W3T_d = dram("W3T", [H, H])
    Wv_d = dram("Wv", [H, 1])
    bih_d = dram("bih", [1, NG])
    bhh_d = dram("bhh", [1, NG])
    wout_d = dram("wout", [H, VPAD])          # per-core slice, zero-padded
    bout_d = dram("boutp", [1, VPAD])         # per-core slice, zero-padded
    sbase_d = dram("sbase", [128, 1])         # per-core k*VSH as fp32
    h0_d = dram("h0", [B, H])

    oseq_d = dram("oseq", [B, T, VPAD], kind="ExternalOutput")
    opred_d = dram("opred", [B, T], i32, kind="ExternalOutput")
    import os as _os
    KDBG = _os.environ.get("KDBG") == "1"
    if KDBG:
        dbg_aw = dram("dbg_aw", [B, S], kind="ExternalOutput")
        dbg_cterm = dram("dbg_cterm", [B, NG], kind="ExternalOutput")
        dbg_ctxT = dram("dbg_ctxT", [128, KH, B], kind="ExternalOutput")
        dbg_h = dram("dbg_h", [B, H], kind="ExternalOutput")
        dbg_w = dram("dbg_w", [B, WD], kind="ExternalOutput")
        dbg_ue = dram("dbg_ue", [128, 4], kind="ExternalOutput")
        dbg_sc = dram("dbg_sc", [B, S], kind="ExternalOutput")

    with tile.TileContext(nc) as tc:
        tc.race_detector_enabled = False
        stack = contextlib.ExitStack()
        live = stack.enter_context(tc.tile_pool(name="live", bufs=1))

        # ---- long-lived state ----
        h_t = live.tile([B, H], f32, name="h_t")
        c_t = live.tile([B, H], f32, name="c_t")
        xT = live.tile([128, KX, B], f32, name="xT")       # [wT(8) ; hT(4)]
        cterm = live.tile([B, NG], f32, name="cterm")
        preds = live.tile([B, T], f32, name="preds")
        wu = live.tile([B, 1], u32, name="wu")
        segbase = live.tile([128, 1], f32, name="segbase")
        ident = live.tile([B, B], f32, name="ident")

        nc.sync.dma_start(h_t[:], h0_d[:])
        nc.vector.memset(c_t[:], 0.0)
        nc.vector.memset(wu[:], 1)          # <BOS> = 1
        nc.vector.memset(preds[:], 0.0)

        nc.sync.dma_start(segbase[:], sbase_d[:])
        nc.vector.tensor_scalar_add(segbase[64:128, :], segbase[64:128, :], 512.0)

        def transpose_into_xT(pool, src, n_blk, xt_off):
            """PE-transpose src [B, n_blk*128] into xT[:, xt_off:xt_off+n_blk, :]."""
            tp = pool.tile([128, n_blk, B], f32, name=f"tpx{xt_off}", tag="tpx")
            for c in range(n_blk):
                nc.tensor.transpose(tp[:, c, :], src[:, c * 128:(c + 1) * 128],
                                    ident[:])
            nc.vector.tensor_copy(xT[:, xt_off:xt_off + n_blk, :], tp[:])

        # =========== preamble ===========
        res = stack.enter_context(tc.tile_pool(name="res", bufs=1))
        brep_fill = res.tile([128, 4, 512], f32, name="brep")
        wout_s = res.tile([128, KH, VSH], f32, name="wout_s")
        with tc.tile_pool(name="pre", bufs=1) as pre, \
             tc.tile_pool(name="prestream", bufs=3) as prestream, \
             tc.tile_pool(name="pps", bufs=1, space="PSUM") as pps:

            # identity for PE transposes
            io_r = pre.tile([B, B], f32, name="io_r")
            io_c = pre.tile([B, B], f32, name="io_c")
            nc.gpsimd.iota(io_r[:], pattern=[[0, B]], base=0, channel_multiplier=1,
                           allow_small_or_imprecise_dtypes=True)
            nc.gpsimd.iota(io_c[:], pattern=[[1, B]], base=0, channel_multiplier=0,
                           allow_small_or_imprecise_dtypes=True)
            nc.vector.tensor_tensor(ident[:], io_r[:], io_c[:], Alu.is_equal)

            transpose_into_xT(pps, h_t, KH, KW)

            # bsr = broadcast(bih) + broadcast(bhh)  (cterm used as scratch)
            brow = pre.tile([1, NG], f32, name="brow", tag="brow")
            bsr = pre.tile([B, NG], f32, name="bsr")
            nc.sync.dma_start(brow[:], bih_d[:])
            nc.gpsimd.partition_broadcast(bsr[:], brow[0:1, :])
            brow2 = pre.tile([1, NG], f32, name="brow2", tag="brow")
            nc.sync.dma_start(brow2[:], bhh_d[:])
            nc.gpsimd.partition_broadcast(cterm[:], brow2[0:1, :])
            nc.vector.tensor_tensor(bsr[:], bsr[:], cterm[:], Alu.add)

            # --- u-chain: u_e = W1e @ (W2 @ (W3 @ Wv)) ---
            wv_t = pre.tile([128, 4, 1], f32, name="wv_t")
            nc.sync.dma_start(wv_t[:], Wv_d.rearrange("(k p) n -> p k n", p=128))

            def matvec(wsrc_d, vin, vout_name):
                wt = pre.tile([128, 4, H], f32, name=f"wt_{vout_name}",
                              tag="wtr", bufs=2)
                nc.sync.dma_start(wt[:],
                                  wsrc_d.rearrange("(k p) n -> p k n", p=128))
                ps = pps.tile([128, 4], f32, name=f"ps_{vout_name}", tag="mv")
                for m in range(4):
                    for k in range(4):
                        nc.tensor.matmul(ps[:, m:m + 1],
                                         wt[:, k, m * 128:(m + 1) * 128],
                                         vin[:, k, :],
                                         start=(k == 0), stop=(k == 3))
                out = pre.tile([128, 4, 1], f32, name=vout_name)
                nc.vector.tensor_copy(out[:, :, 0], ps[:])
                return out

            v3 = matvec(W3T_d, wv_t, "v3")
            v2 = matvec(W2T_d if False else W2T_d, v3, "v2")
            ue = matvec(W1eT_d, v2, "ue")

            # --- scores -> sc_bs [B, S]: u_e^T @ encT, streamed ---
            sc_bs = pre.tile([B, S], f32, name="sc_bs")
            for j in range((B * S) // 512):
                et = prestream.tile([128, 4, 512], f32, name="et", tag="et")
                nc.sync.dma_start(
                    et[:], encT_d.rearrange("(k p) n -> p k n", p=128)[
                        :, :, j * 512:(j + 1) * 512])
                sp = pps.tile([1, 512], f32, name="sp", tag="sp")
                for k in range(4):
                    nc.tensor.matmul(sp[:], ue[:, k, :], et[:, k, :],
                                     start=(k == 0), stop=(k == 3))
                stg1 = prestream.tile([1, 2, 256], f32, name="stg1", tag="stg1")
                nc.vector.tensor_copy(stg1[:], sp.rearrange("o (b s) -> o b s", b=2))
                for bb in range(2):
                    nc.sync.dma_start(sc_bs[2 * j + bb:2 * j + bb + 1, :],
                                      stg1[:, bb, :])

            if KDBG:
                nc.sync.dma_start(dbg_sc[:], sc_bs[:])
            # --- softmax rows -> aw [B, S] ---
            rmax = pre.tile([B, 1], f32, name="rmax")
            nc.vector.reduce_max(rmax[:], sc_bs[:], axis=Axis.X)
            nc.vector.tensor_scalar(sc_bs[:], sc_bs[:], rmax[:], None,
                                    op0=Alu.subtract)
            nc.scalar.activation(sc_bs[:], sc_bs[:], Act.Exp)
            rsum = pre.tile([B, 1], f32, name="rsum")
            nc.vector.reduce_sum(rsum[:], sc_bs[:], axis=Axis.X)
            rinv = pre.tile([B, 1], f32, name="rinv")
            nc.vector.reciprocal(rinv[:], rsum[:])
            nc.vector.tensor_scalar(sc_bs[:], sc_bs[:], rinv[:], None,
                                    op0=Alu.mult)

            if KDBG:
                nc.sync.dma_start(dbg_aw[:], sc_bs[:])
                nc.sync.dma_start(dbg_ue[:], ue[:, :, 0])
            # --- awT [S, B] via PE transpose ---
            awT = pre.tile([128, 2, B], f32, name="awT")
            tpa = pps.tile([128, 2, B], f32, name="tpa", tag="tpx")
            for scn in range(2):
                nc.tensor.transpose(tpa[:, scn, :],
                                    sc_bs[:, scn * 128:(scn + 1) * 128],
                                    ident[:])
            nc.vector.tensor_copy(awT[:], tpa[:])

            # --- ctxT [H, B] ---
            ctxT = pre.tile([128, KH, B], f32, name="ctxT")
            cps = [pps.tile([128, B], f32, name=f"cps{hc}", tag=f"cps{hc}")
                   for hc in range(KH)]
            for b in range(B):
                eb = prestream.tile([128, 2, H], f32, name="eb", tag="eb")
                nc.sync.dma_start(
                    eb[:],
                    enc_d.rearrange("(b sc p) h -> b p sc h", b=B, sc=2)[b])
                for hc in range(KH):
                    for scn in range(2):
                        nc.tensor.matmul(cps[hc][:, b:b + 1],
                                         eb[:, scn, hc * 128:(hc + 1) * 128],
                                         awT[:, scn, b:b + 1],
                                         start=(scn == 0), stop=(scn == 1))
            for hc in range(KH):
                nc.vector.tensor_copy(ctxT[:, hc, :], cps[hc][:])
            if KDBG:
                nc.sync.dma_start(dbg_ctxT[:], ctxT[:])

            # --- cterm = ctx @ Wc + (bih + bhh) ---
            wcv = Wc_d.rearrange("(k p) n -> p k n", p=128)
            for vg in range(4):
                for hf in range(2):
                    bsb = pre.tile([1, 512], f32, name=f"bsb{vg}{hf}",
                                   tag="bsb", bufs=2)
                    nc.sync.dma_start(
                        bsb[:], bout_d[0:1, 1024 * vg + 512 * hf:
                                       1024 * vg + 512 * hf + 512])
                    bstg = pre.tile([B, 512], f32, name=f"bstg{vg}{hf}",
                                    tag="bstg", bufs=2)
                    nc.gpsimd.partition_broadcast(bstg[:], bsb[0:1, :])
                    if hf == 0:
                        nc.vector.tensor_copy(brep_fill[0:64, vg, :], bstg[:])
                    else:
                        nc.sync.dma_start(brep_fill[64:128, vg, :], bstg[:])
            for nb in range(4):
                wct = pre.tile([128, KH, 512], f32, name=f"wct{nb}",
                               tag="wct", bufs=2)
                nc.sync.dma_start(wct[:], wcv[:, :, nb * 512:(nb + 1) * 512])
                ctp = pps.tile([B, 512], f32, name=f"ctp{nb}", tag="ctp")
                for k in range(KH):
                    nc.tensor.matmul(ctp[:], ctxT[:, k, :],
                                     wct[:, k, :],
                                     start=(k == 0), stop=(k == KH - 1))
                nc.vector.tensor_tensor(cterm[:, nb * 512:(nb + 1) * 512],
                                        ctp[:], bsr[:, nb * 512:(nb + 1) * 512],
                                        Alu.add)

        if KDBG:
            nc.sync.dma_start(dbg_cterm[:], cterm[:])
        # =========== resident weights ===========
        brep = brep_fill
        nc.sync.dma_start(
            wout_s[:],
            wout_d.rearrange("(k p) n -> p k n", p=128)[:, :, 0:VSH])

        stepdram = stack.enter_context(
            tc.tile_pool(name="sd", bufs=2, space="DRAM"))
        work = stack.enter_context(tc.tile_pool(name="work", bufs=1))
        wk2 = stack.enter_context(tc.tile_pool(name="wk2", bufs=2))
        psg = stack.enter_context(tc.tile_pool(name="psg", bufs=1, space="PSUM"))

        # =========== decode steps ===========
        for t in range(T):
            # --- gather w = emb[wu] ---
            wrow = work.tile([B, WD], f32, name="wrow", tag="wrow")
            nc.gpsimd.indirect_dma_start(
                out=wrow[:], out_offset=None, in_=emb_d[:],
                in_offset=bass.IndirectOffsetOnAxis(ap=wu[:], axis=0))
            if KDBG and t == 0:
                nc.sync.dma_start(dbg_w[:], wrow[:])
            transpose_into_xT(psg, wrow, KW, 0)

            # --- gates: 2 bank groups, col-tiled pairs ---
            gp = [psg.tile([128, 512], f32, name=f"gp{bg}", tag=f"gp{bg}")
                  for bg in range(2)]
            wpv = Wp_d.rearrange("(k p) n -> p k n", p=128)
            for bg in range(2):
                for k in range(KX):
                    wpt = wk2.tile([128, 1024], f32, name="wpt", tag="wpt",
                                   bufs=4)
                    nc.sync.dma_start(wpt[:],
                                      wpv[:, k, 1024 * bg:1024 * (bg + 1)])
                    nc.tensor.matmul(gp[bg][0:64, :], xT[:, k, :],
                                     wpt[:, 0:512],
                                     start=(k == 0), stop=(k == KX - 1),
                                     tile_position=(0, 0))
                    nc.tensor.matmul(gp[bg][64:128, :], xT[:, k, :],
                                     wpt[:, 512:1024],
                                     start=(k == 0), stop=(k == KX - 1),
                                     tile_position=(0, 64))
            # PSUM upper halves (gates f, o) -> SBUF -> shift down to 0:64
            stg = work.tile([128, 512], f32, name="stg", tag="stg")
            fsh = work.tile([B, 512], f32, name="fsh", tag="fsh")
            osh = work.tile([B, 512], f32, name="osh", tag="osh")
            nc.vector.tensor_copy(stg[64:128, :], gp[0][64:128, :])
            nc.sync.dma_start(fsh[:], stg[64:128, :])
            nc.vector.tensor_copy(stg[0:64, :], gp[1][64:128, :])
            nc.sync.dma_start(osh[:], stg[0:64, :])

            # --- LSTM elementwise (fa in fsh, oa in osh, in place) ---
            ia = work.tile([B, 512], f32, name="ia", tag="ia")
            ga = work.tile([B, 512], f32, name="ga", tag="ga")
            nc.vector.tensor_tensor(ia[:], gp[0][0:64, :], cterm[:, 0:512],
                                    Alu.add)
            nc.vector.tensor_tensor(fsh[:], fsh[:], cterm[:, 512:1024], Alu.add)
            nc.vector.tensor_tensor(ga[:], gp[1][0:64, :], cterm[:, 1024:1536],
                                    Alu.add)
            nc.vector.tensor_tensor(osh[:], osh[:], cterm[:, 1536:2048], Alu.add)
            nc.scalar.activation(ia[:], ia[:], Act.Sigmoid)
            nc.scalar.activation(fsh[:], fsh[:], Act.Sigmoid)
            nc.scalar.activation(ga[:], ga[:], Act.Tanh)
            nc.scalar.activation(osh[:], osh[:], Act.Sigmoid)
            nc.vector.tensor_tensor(fsh[:], fsh[:], c_t[:], Alu.mult)
            nc.vector.tensor_tensor(ia[:], ia[:], ga[:], Alu.mult)
            nc.vector.tensor_tensor(c_t[:], fsh[:], ia[:], Alu.add)
            nc.scalar.activation(ga[:], c_t[:], Act.Tanh)
            nc.vector.tensor_tensor(h_t[:], osh[:], ga[:], Alu.mult)

            if KDBG and t == 0:
                nc.sync.dma_start(dbg_h[:], h_t[:])
            transpose_into_xT(psg, h_t, KH, KW)

            # --- logits: 4 groups, col-tiled pairs; argmax piggyback ---
            candv = work.tile([128, 4, 8], f32, name="candv", tag="candv")
            candg = work.tile([128, 4, 8], f32, name="candg", tag="candg")
            mi = work.tile([128, 8], u32, name="mi", tag="mi")
            for vg in range(4):
                loL, nL, loR, nR = VG_RANGES[vg]
                lp = psg.tile([128, 512], f32, name=f"lp{vg}", tag=f"lp{vg}")
                for k in range(KH):
                    nc.tensor.matmul(lp[0:64, 0:nL], xT[:, KW + k, :],
                                     wout_s[:, k, loL:loL + nL],
                                     start=(k == 0), stop=(k == KH - 1),
                                     tile_position=(0, 0))
                    nc.tensor.matmul(lp[64:128, 0:nR], xT[:, KW + k, :],
                                     wout_s[:, k, loR:loR + nR],
                                     start=(k == 0), stop=(k == KH - 1),
                                     tile_position=(0, 64))
                lg = wk2.tile([128, 512], f32, name=f"lg{vg}", tag="lg")
                nc.vector.tensor_tensor(lg[:], lp[:], brep[:, vg, :], Alu.add)
                nc.sync.dma_start(oseq_d[:, t, loL:loL + nL], lg[0:64, 0:nL])
                nc.sync.dma_start(oseq_d[:, t, loR:loR + nR], lg[64:128, 0:nR])
                # per-bank top8 (both partition halves at once where possible)
                nc.vector.max(candv[0:64, vg, :], lg[0:64, 0:nL])
                nc.vector.max_index(mi[0:64, :], candv[0:64, vg, :],
                                    lg[0:64, 0:nL])
                nc.vector.max(candv[64:128, vg, :], lg[64:128, 0:nR])
                nc.vector.max_index(mi[64:128, :], candv[64:128, vg, :],
                                    lg[64:128, 0:nR])
                # global idx = idx + vg_base + segbase(+512 upper) + core base
                nc.vector.tensor_copy(candg[:, vg, :], mi[:])   # u32 -> f32
                nc.vector.tensor_scalar(candg[:, vg, :], candg[:, vg, :],
                                        segbase[:], float(1024 * vg),
                                        op0=Alu.add, op1=Alu.add)

            # --- local winner over 32 candidates per partition ---
            cv = candv.rearrange("p a c -> p (a c)")
            cg = candg.rearrange("p a c -> p (a c)")
            vmax = work.tile([128, 1], f32, name="vmax", tag="vmax")
            nc.vector.reduce_max(vmax[:], cv[:], axis=Axis.X)
            msk = work.tile([128, 32], f32, name="msk", tag="msk")
            nc.vector.tensor_scalar(msk[:], cv[:], vmax[:], None,
                                    op0=Alu.is_equal)
            sel = work.tile([128, 32], f32, name="sel", tag="sel")
            nc.vector.tensor_tensor(sel[:], cg[:], msk[:], Alu.mult)
            nc.vector.tensor_scalar(msk[:], msk[:], -1e9, 1e9,
                                    op0=Alu.mult, op1=Alu.add)
            nc.vector.tensor_tensor(sel[:], sel[:], msk[:], Alu.add)
            wg = work.tile([128, 1], f32, name="wg", tag="wg")
            nc.vector.tensor_reduce(wg[:], sel[:], axis=Axis.X, op=Alu.min)

            # merge partition halves -> [B, 2] candidates
            pv = work.tile([B, 2], f32, name="pv", tag="pv")
            pg = work.tile([B, 2], f32, name="pg", tag="pg")
            nc.vector.tensor_copy(pv[:, 0:1], vmax[0:64, :])
            nc.vector.tensor_copy(pg[:, 0:1], wg[0:64, :])
            nc.sync.dma_start(pv[:, 1:2], vmax[64:128, :])
            nc.sync.dma_start(pg[:, 1:2], wg[64:128, :])

            def merge(into_g, vals, gidx, n, tag):
                vm = work.tile([B, 1], f32, name=f"vm_{tag}", tag=f"vm{tag}")
                nc.vector.reduce_max(vm[:], vals[:], axis=Axis.X)
                mk = work.tile([B, n], f32, name=f"mk_{tag}", tag=f"mk{tag}")
                nc.vector.tensor_scalar(mk[:], vals[:], vm[:], None,
                                        op0=Alu.is_equal)
                s1 = work.tile([B, n], f32, name=f"s1_{tag}", tag=f"s1{tag}")
                nc.vector.tensor_tensor(s1[:], gidx[:], mk[:], Alu.mult)
                nc.vector.tensor_scalar(mk[:], mk[:], -1e9, 1e9,
                                        op0=Alu.mult, op1=Alu.add)
                nc.vector.tensor_tensor(s1[:], s1[:], mk[:], Alu.add)
                nc.vector.tensor_reduce(into_g[:], s1[:], axis=Axis.X,
                                        op=Alu.min)
                return vm

            lgx = work.tile([B, 1], f32, name="lgx", tag="lgx")
            lv = merge(lgx, pv, pg, 2, "h")

            # --- cross-core exchange: AllGather of (val, gidx) ---
            xpk = work.tile([B, 2], f32, name="xpk", tag="xpk")
            nc.vector.tensor_copy(xpk[:, 0:1], lv[:])
            nc.vector.tensor_copy(xpk[:, 1:2], lgx[:])
            bin_t = stepdram.tile([B, 2], f32, name="bin_t", tag="bin_t")
            bout_t = stepdram.tile([N_CORES * B, 2], f32, name="bout_t",
                                   tag="bout_t")
            nc.gpsimd.dma_start(bin_t[:], xpk[:])
            nc.gpsimd.collective_compute(
                "AllGather", Alu.bypass,
                replica_groups=[list(range(N_CORES))],
                ins=[bin_t.opt()], outs=[bout_t.opt()])
            gat = work.tile([B, N_CORES, 2], f32, name="gat", tag="gat")
            nc.gpsimd.dma_start(
                gat[:], bout_t.rearrange("(k b) e -> b k e", k=N_CORES))
            gv = work.tile([B, N_CORES], f32, name="gv", tag="gv")
            gg = work.tile([B, N_CORES], f32, name="gg", tag="gg")
            nc.vector.tensor_copy(gv[:], gat[:, :, 0])
            nc.vector.tensor_copy(gg[:], gat[:, :, 1])
            wfin = work.tile([B, 1], f32, name="wfin", tag="wfin")
            merge(wfin, gv, gg, N_CORES, "c")

            nc.vector.tensor_copy(preds[:, t:t + 1], wfin[:])
            nc.vector.tensor_copy(wu[:], wfin[:])       # f32 -> u32 cast

        # --- predictions out ---
        pi = live.tile([B, T], i32, name="pi")
        nc.vector.tensor_copy(pi[:], preds[:])
        nc.sync.dma_start(opred_d[:], pi[:])
        stack.close()

    nc.compile()
    return nc


def _prep_inputs(inputs):
    def gi(k):
        return np.ascontiguousarray(np.asarray(inputs[k], dtype=np.float32))

    enc = gi("encoder_output")                        # [B, S, H]
    emb = gi("emb")
    W1 = gi("W1"); W2 = gi("W2"); W3 = gi("W3"); Wv = gi("Wv")
    Wih = gi("Wih"); Whh = gi("Whh")
    bih = gi("bih"); bhh = gi("bhh")
    Wout = gi("Wout"); bout = gi("bout")
    h0 = gi("encoder_last_hidden_state")[0]           # [B, H]

    enc_flat = np.ascontiguousarray(enc.reshape(B * S, H))
    encT = np.ascontiguousarray(enc_flat.T)
    Wp = np.ascontiguousarray(np.concatenate([Wih[:WD], Whh], axis=0))
    Wc = np.ascontiguousarray(Wih[WD:WD + H])
    W1eT = np.ascontiguousarray(W1[:H, :].T)
    W2T = np.ascontiguousarray(W2.T)
    W3T = np.ascontiguousarray(W3.T)

    in_maps = []
    for k in range(N_CORES):
        wsl = np.zeros((H, VPAD), np.float32)
        bsl = np.zeros((1, VPAD), np.float32)
        wsl[:, :VSH] = Wout[:, k * VSH:(k + 1) * VSH]
        bsl[0, :VSH] = bout[k * VSH:(k + 1) * VSH]
        in_maps.append({
            "emb": emb,
            "enc": enc_flat,
            "encT": encT,
            "Wp": Wp,
            "Wc": Wc,
            "W1eT": W1eT,
            "W2T": W2T,
            "W3T": W3T,
            "Wv": np.ascontiguousarray(Wv.reshape(H, 1)),
            "bih": np.ascontiguousarray(bih.reshape(1, NG)),
            "bhh": np.ascontiguousarray(bhh.reshape(1, NG)),
            "wout": np.ascontiguousarray(wsl),
            "boutp": np.ascontiguousarray(bsl),
            "sbase": np.full((128, 1), float(k * VSH), np.float32),
            "h0": np.ascontiguousarray(h0),
        })
    return in_maps


def kernel(**inputs):
    from concourse.bass_utils import run_bass_kernel_spmd

    T = int(np.asarray(inputs["targets"]).shape[1]) - 1
    if T not in _CACHE:
        _CACHE[T] = _build(T)
    nc = _CACHE[T]
    in_maps = _prep_inputs(inputs)
    res = run_bass_kernel_spmd(nc, in_maps, core_ids=list(range(N_CORES)))
    seq = np.zeros((B, T, V), np.float32)
    for k in range(N_CORES):
        seq[:, :, k * VSH:(k + 1) * VSH] = res.results[k]["oseq"][:, :, 0:VSH]
    preds = res.results[0]["opred"].astype(np.int32)
    return seq, preds
